# revision 35
# baseline (speedup 1.0000x reference)
"""Trainium2 Bass kernel for a dense transformer block (RMSNorm + MHA + SwiGLU MLP).

Sharding: sequence-parallel over the 8 cores (batch 0 -> cores 0-3,
batch 1 -> cores 4-7; each core owns 512 tokens).  Each core computes
q/k/v for its own tokens, the K/V shards are exchanged with chunked
merged-K+V AllGathers, and everything else (attention for the local
queries, o-proj, MLP) is computed locally with full (replicated)
weights streamed from HBM.

The serial collective chain is the kernel's critical path outside the
MLP (each AllGather costs ~15us of ncfw control latency plus wire
time), so the design centers on it:
 - K and V of a chunk travel in ONE fp8 AllGather (8 collectives -> 4),
   laid out so each head's K row | V row is a contiguous 1KB-per-
   partition block gathered by a single DMA.
 - Chunk sizes [4,6,4,2] (sim-swept): the chain's end time is fixed by
   total bytes + per-collective latency, so the schedule buys an early
   exp start (first chunk small-ish) and a short post-chain tail (last
   chunk tiny).
 - Everything before the first collective is latency-tuned: projections
   run on raw fp8 x (x8 = x/16, quantized straight off the incoming
   DMA with no rmsnorm dependency), and the per-token 1/rms folds into
   the PSUM->SBUF copies (K/Q: rinv1 muls on the vector engine; V,
   whose PSUM has tokens on partitions, uses a PE-transposed rinv as a
   per-partition activation scale).  The rmsnorm square-sums alternate
   vector/gpsimd so neither engine alone paces the chain.
 - Per-head K/V loads issue from the scalar queue: on the gpsimd queue
   they would serialize behind ALL later chunks' collectives.
 - All 16 Q projections are hoisted before the attention loop (the PE
   queue is in-order: issued per-head they head-of-line block behind
   collective-gated scores matmuls).

Precision: fp8-e4m3 with DoubleRow matmuls for q/k/v projections,
softmax denominator, attn@V and the o-projection; QK^T scores are fp8
(non-DR; unit-scale q/k, measured +0.5e-3 total error).  The MLP stays
bf16: measured CPU-emulated error for fp8 gate/up is 3.6e-2, over the
2e-2 budget.  Matmul accumulation is always fp32 in PSUM.

Scaling: fp8-e4m3 flushes to subnormals below 2^-6 and weights have
std 0.02, so attention weights carry 64x and x carries 64x host-side
(the residual stream stays at 64x; rmsnorm is scale-invariant).  x8
carries 4x, so rinv1 = 1/(256*rms) makes K/Q/V unit-scale fp8;
exp(qk/sqrt(hd) - 4) needs no further descale (-4 keeps e under fp8's
240 max, which would overflow to Inf on TRN, and doubles as a max-free
softmax shift: scores are O(5) here).  wd carries 64x in bf16 (exact)
so the MLP output matches the 64x residual; the host divides the final
output by 64.

Attention is software-pipelined one head ahead (scores/exp of head h+1
issue before denominator/AV of head h) so the PE never waits for the
scalar engine's exp, the critical resource of the attention phase.

Timing (TimelineSim cost model, calibrated +0.76% vs the harness on
the 1220us baseline): 1040us predicted; PE busy ~78%, MLP phase at
~97% of the bf16 roofline.
"""

import math

import ml_dtypes
import numpy as np

import concourse.bass as bass
import concourse.mybir as mybir
import concourse.tile as tile
from concourse import bacc
from concourse.bass import ts
from concourse.bass_utils import run_bass_kernel_spmd

F32 = mybir.dt.float32
BF16 = mybir.dt.bfloat16
F8 = mybir.dt.float8e4

B = 2
T = 2048
C = 2048  # hidden
I = 8192  # intermediate
NH = 16
HD = 128
EPS = 1e-6

N_CORES = 8
GROUP = 4  # cores per batch group
T_LOC = (B * T) // N_CORES  # 512 tokens per core
GROUPS = [[0, 1, 2, 3], [4, 5, 6, 7]]

P = 128
NCT = C // P  # 16 c-tiles
NCP = NCT // 2  # 8 c-tile pairs (DoubleRow)
NIT = I // P  # 64 i-tiles
NTT = T_LOC // P  # 4 local token tiles
NKT = T // P  # 16 key tiles per batch
NKP = NKT // 2  # 8 key-tile pairs
IHALF = NIT // 2  # 32 i-tiles per MLP half
NCHUNK = 4  # collective chunks
HPC = NH // NCHUNK  # max heads per chunk (kv buffer sizing)
CHUNK_SIZES = [4, 6, 4, 2]  # heads per chunk (sim-swept: early exp start vs short last-chunk tail)

WS = 64.0  # host-side weight/x scale (2^6)
DR = mybir.MatmulPerfMode.DoubleRow

_CACHE = {}


def _pack_lhsT(w, dtype):
    """[O, C] weight (y = x @ w.T) -> [O//128, 128, C] of stationary tiles.

    block[ot][ci, ct*128 + oi] == w[128*ot + oi, 128*ct + ci], so the
    SBUF tile [128, C] for output-tile `ot` yields lhsT slices
    [:, ct*128 : ct*128+128] = w.T tile with contraction on partitions.
    """
    O, Cin = w.shape
    no, nc_ = O // P, Cin // P
    arr = w.reshape(no, P, nc_, P).transpose(0, 3, 2, 1)  # [ot, ci, ct, oi]
    return np.ascontiguousarray(arr.reshape(no, P, Cin)).astype(dtype)


def _build_program(sim_mode=False, unroll=1, nchunk=NCHUNK):
    nc = bacc.Bacc("TRN2", target_bir_lowering=False, debug=False, num_devices=N_CORES)
    global NCHUNK, HPC, CHUNK_SIZES
    NCHUNK, HPC = nchunk, NH // nchunk
    if nchunk == 4:
        sizes = CHUNK_SIZES
    else:
        sizes = [NH // nchunk] * nchunk
    starts = [sum(sizes[:i]) for i in range(len(sizes))]
    maxh = max(sizes)
    ck_of = {}
    oi_of = {}
    for ci, (s0, n) in enumerate(zip(starts, sizes)):
        for j in range(n):
            ck_of[s0 + j] = ci
            oi_of[s0 + j] = j

    x_d = nc.declare_dram_parameter("x", [C, T_LOC], F32, isOutput=False)
    wqT_d = nc.declare_dram_parameter("wqT", [NCT, P, C], F8, isOutput=False)
    wkT_d = nc.declare_dram_parameter("wkT", [NCT, P, C], F8, isOutput=False)
    wv8_d = nc.declare_dram_parameter("wv8", [P, NCT, C], F8, isOutput=False)
    woT_d = nc.declare_dram_parameter("woT", [NCT, P, C], F8, isOutput=False)
    wgT_d = nc.declare_dram_parameter("wgT", [NIT, P, C], BF16, isOutput=False)
    wuT_d = nc.declare_dram_parameter("wuT", [NIT, P, C], BF16, isOutput=False)
    wdT_d = nc.declare_dram_parameter("wdT", [NCT, P, I], BF16, isOutput=False)
    ident_d = nc.declare_dram_parameter("ident", [P, P], F32, isOutput=False)
    out_d = nc.declare_dram_parameter("out", [C, T_LOC], F32, isOutput=True)

    # merged K+V collective bounce buffers, one AllGather per 4-head chunk
    # (8 collectives -> 4: each costs ~15us of ncfw latency on top of the
    # wire time, so halving the count cuts ~60us off the serial collective
    # chain).  Everything fp8: K blocks are feature-major [head][hd, t] (the
    # scores matmul is fp8 now), V blocks token-major [head][t, hd].
    # layout: [chunk][head][sbuf-partition][K row | V row] where the K row
    # is the head's 512 token scores-operand bytes and the V row its 512
    # (tt,d)-flattened AV-operand bytes -> each head gathers with ONE DMA of
    # 1KB-contiguous per-partition segments (>=512B avoids the DMA
    # read-modify-write penalty).
    kv_cc = [
        nc.dram_tensor(f"kv_cc{ci}", [n, P, 2, T_LOC], F8)
        for ci, n in enumerate(sizes)
    ]
    kv_ag = [
        nc.dram_tensor(f"kv_ag{ci}", [GROUP, n, P, 2, T_LOC], F8)
        for ci, n in enumerate(sizes)
    ]

    def allgather(in_ap, out_ap):
        if sim_mode:
            for g in range(GROUP):
                nc.gpsimd.dma_start(out=out_ap[g], in_=in_ap)
        else:
            nc.gpsimd.collective_compute(
                "AllGather", mybir.AluOpType.bypass, replica_groups=GROUPS,
                ins=[in_ap], outs=[out_ap],
            )

    isq = 1.0 / math.sqrt(HD)

    with tile.TileContext(nc) as tc:
        with (
            tc.tile_pool(name="sb", bufs=2) as sb,
            tc.tile_pool(name="ps", bufs=2, space="PSUM") as ps,
        ):
            ones_bf = sb.tile([P, P], BF16, tag="ones_bf", bufs=1)
            nc.any.memset(ones_bf, 1.0)
            ones8 = sb.tile([P, 2, P], F8, tag="ones8", bufs=1)
            nc.any.memset(ones8, 1.0)
            eps_t = sb.tile([P, 1], F32, tag="eps", bufs=1)
            nc.any.memset(eps_t, EPS * WS * WS)  # sqrt(ssq/C + 4096eps) = 64*rms
            eps16_t = sb.tile([P, 1], F32, tag="eps16", bufs=1)
            nc.any.memset(eps16_t, EPS * WS * WS * 16.0)
            ebias_t = sb.tile([P, 1], F32, tag="ebias", bufs=1)
            nc.any.memset(ebias_t, -4.0)
            ident = sb.tile([P, P], F32, tag="ident", bufs=1)
            nc.sync.dma_start(out=ident[:], in_=ident_d[:, :])

            for _rep in range(unroll):
                # ---- load x (pre-scaled by 64 on host) ----
                x_sb = sb.tile([P, NCT, T_LOC], F32, tag="t32", bufs=1, name="x_sb")
                x_d_v = x_d.rearrange("(ct p) t -> p ct t", p=P)
                # per-tile x DMAs: the sq/x8 pipeline consumes tiles as they
                # land (a single big DMA stalls everything ~11us)
                for ct in range(NCT):
                    nc.sync.dma_start(out=x_sb[:, ct, :], in_=x_d_v[:, ct, :])

                def rms_rinv(src_sb, eps_tile, scale):
                    # squares in bf16 (ones-matmul then runs 1 cycle/row); sq
                    # muls alternate vector/gpsimd (one DVE alone paces this
                    # ~10us and delays the first collective launch 1:1).
                    ssq = ps.tile([P, T_LOC], F32, tag="dn", name="ssq")
                    for ct in range(NCT):
                        sq = sb.tile([P, T_LOC], BF16, tag="tmp", bufs=3, name="sq")
                        eng = nc.gpsimd if ct < 5 else nc.vector
                        eng.tensor_mul(sq[:], src_sb[:, ct, :], src_sb[:, ct, :])
                        nc.tensor.matmul(
                            ssq[:], ones_bf[:], sq[:], start=(ct == 0), stop=(ct == NCT - 1)
                        )
                    rms = sb.tile([P, T_LOC], F32, tag="tmp", bufs=3, name="rms")
                    nc.scalar.activation(
                        rms[:], ssq[:], mybir.ActivationFunctionType.Sqrt,
                        bias=eps_tile[:], scale=scale,
                    )
                    rinv = sb.tile([P, T_LOC], F32, tag="rinv", bufs=2, name="rinv")
                    nc.vector.reciprocal(rinv[:], rms[:])
                    return rinv

                # raw-x8 projections: quantize x straight to fp8 (no rms
                # dependency -> projections start ~8us earlier) and fold the
                # per-token 1/rms into the PSUM->SBUF copies instead.  The
                # norm1 sqrt is pre-scaled by 16 so rinv1 = 1/(256*rms)
                # exactly cancels x8's 4x and the weights' 64x.
                x8 = sb.tile([P, NCP, 2, T_LOC], F8, tag="xn", bufs=1, name="x8")
                for ct in range(NCT):
                    nc.scalar.activation(
                        x8[:, ct // 2, ct % 2, :], x_sb[:, ct, :],
                        mybir.ActivationFunctionType.Copy, scale=1.0 / 16.0,
                    )
                rinv1 = rms_rinv(x_sb, eps16_t, 16.0 / C)
                # V's matmul puts tokens on PSUM partitions, where rinv1's
                # per-token free-dim layout can't broadcast.  PE-transpose
                # rinv1 (its rows are identical) into rv[p, tt] =
                # 1/(256*rms(token tt*128+p)); the V psum->sbuf copies then
                # use it as a per-partition activation scale.  (The PE is
                # idle here; a DRAM-bounce transpose would clog the shared
                # DMA pool right when the K/V bounce writes need it.)
                rv_sb = sb.tile([P, NTT], F32, tag="rv", bufs=1, name="rv_sb")
                for tt in range(NTT):
                    rv_ps = ps.tile([P, P], F32, tag="mm", bufs=4, name="rv_ps")
                    nc.tensor.transpose(rv_ps[:], rinv1[:, ts(tt, P)], ident[:])
                    nc.vector.tensor_copy(rv_sb[:, tt : tt + 1], rv_ps[:, 0:1])

                # ---- K+V projections, interleaved per chunk; ONE merged ----
                # ---- K+V AllGather per chunk (the ~15us ncfw latency    ----
                # ---- dominates at these sizes, so fewer+bigger wins).   ----
                # ---- K: feature-major fp8 (unit scale, for fp8 scores); ----
                # ---- V: token-major (x8 blocks stationary), fp8 out.    ----
                for ck in range(NCHUNK):
                    nh = sizes[ck]
                    k_sb = sb.tile([P, maxh, T_LOC], F8, tag="kst", bufs=2, name="k_sb")
                    for oi in range(nh):
                        ot = starts[ck] + oi
                        wk_t = sb.tile(
                            [P, NCP, 2, P], F8, tag="wqk", bufs=3, name="wk_t"
                        )
                        nc.sync.dma_start(
                            out=wk_t[:],
                            in_=wkT_d[ot].rearrange("p (cp two f) -> p cp two f", two=2, f=P),
                        )
                        k_ps = ps.tile([P, T_LOC], F32, tag="mm", bufs=4, name="k_ps")
                        for cp in range(NCP):
                            nc.tensor.matmul(
                                k_ps[:], wk_t[:, cp, :, :], x8[:, cp, :, :],
                                start=(cp == 0), stop=(cp == NCP - 1), perf_mode=DR,
                            )
                        # per-token 1/(256*rms) -> unit-scale fp8 K
                        # (vector only: gpsimd cannot read PSUM)
                        nc.vector.tensor_mul(k_sb[:, oi, :], k_ps[:], rinv1[:])

                    wv_t = sb.tile(
                        [P, NCP, 2, maxh * HD], F8, tag="wvs", bufs=1, name="wv_t"
                    )
                    nc.sync.dma_start(
                        out=wv_t[:, :, :, : nh * HD],
                        in_=wv8_d[:, :, starts[ck] * HD : (starts[ck] + nh) * HD].rearrange(
                            "p (cp two) o -> p cp two o", two=2
                        ),
                    )
                    # v_sb is head-major so the bounce write is ONE DMA of
                    # 512B-contiguous segments (no sub-512B DMA penalty)
                    v_sb = sb.tile([P, maxh, NTT, HD], F8, tag="vst", bufs=2, name="v_sb")
                    for oh0 in range(0, nh * HD, 512):
                        w = min(512, nh * HD - oh0)
                        v_ps = [
                            ps.tile([P, 512], F32, tag="acc", bufs=2, name="v_ps0"),
                            ps.tile([P, 512], F32, tag="acc", bufs=2, name="v_ps1"),
                            ps.tile([P, 512], F32, tag="dn", bufs=2, name="v_ps2"),
                            ps.tile([P, 512], F32, tag="dn", bufs=2, name="v_ps3"),
                        ]
                        for cp in range(NCP):
                            for tt in range(NTT):
                                nc.tensor.matmul(
                                    v_ps[tt][:, :w],
                                    x8[:, cp, :, ts(tt, P)],
                                    wv_t[:, cp, :, oh0 : oh0 + w],
                                    start=(cp == 0), stop=(cp == NCP - 1), perf_mode=DR,
                                )
                        for tt in range(NTT):
                            # per-token 1/(256*rms) as per-partition scale
                            nc.scalar.activation(
                                v_sb[:, oh0 // HD : (oh0 + w) // HD, tt, :],
                                v_ps[tt][:, :w],
                                mybir.ActivationFunctionType.Copy,
                                scale=rv_sb[:, tt : tt + 1],
                            )
                    nc.gpsimd.dma_start(
                        out=kv_cc[ck][:, :, 0, :].rearrange("hh p t -> p hh t"),
                        in_=k_sb[:, :nh, :],
                    )
                    nc.gpsimd.dma_start(
                        out=kv_cc[ck][:, :, 1, :].rearrange("hh p (tt d) -> p hh tt d", d=HD),
                        in_=v_sb[:, :nh, :, :],
                    )
                    allgather(kv_cc[ck][:], kv_ag[ck][:])

                q_all = sb.tile([P, NH, T_LOC], F8, tag="q", bufs=1, name="q_all")

                def q_proj(ot):
                    wq_t = sb.tile([P, NCP, 2, P], F8, tag="wqk", bufs=3, name="wq_t")
                    nc.sync.dma_start(
                        out=wq_t[:],
                        in_=wqT_d[ot].rearrange("p (cp two f) -> p cp two f", two=2, f=P),
                    )
                    q_ps = ps.tile([P, T_LOC], F32, tag="mm", bufs=4, name="q_ps")
                    for cp in range(NCP):
                        nc.tensor.matmul(
                            q_ps[:], wq_t[:, cp, :, :], x8[:, cp, :, :],
                            start=(cp == 0), stop=(cp == NCP - 1), perf_mode=DR,
                        )
                    # per-token 1/(256*rms) -> unit-scale fp8 Q (vector, so
                    # the scalar engine stays pure-exp during attention)
                    nc.vector.tensor_mul(q_all[:, ot, :], q_ps[:], rinv1[:])

                # ---- attention, software-pipelined one head ahead ----
                # scores/exp of head h+1 are issued before denominator/AV of
                # head h so the PE isn't blocked on the scalar engine's exp.
                # k_h/v_h loads go through the SCALAR queue: the gpsimd
                # queue is occupied by the serial collective chain (a load
                # queued there would wait for ALL previous chunks'
                # collectives, not just its own), and the sync queue carries
                # the weight streams.  The only scalar work ever queued
                # behind a waiting load is the same chunk's exp, which
                # depends on the load anyway.
                def load_head(h):
                    kv_h = sb.tile(
                        [P, GROUP, 2, NTT, HD], F8, tag="kvh", bufs=2, name="kv_h"
                    )
                    nc.scalar.dma_start(
                        out=kv_h[:],
                        in_=kv_ag[ck_of[h]][:, oi_of[h]].rearrange(
                            "g p kv t -> p g kv t"
                        ).rearrange("p g kv (tt d) -> p g kv tt d", d=HD),
                    )
                    return kv_h

                def scores_exp(h, kv_h):
                    # e = exp(q.k/sqrt(hd) - 4); q/k are unit-scale fp8.  -4
                    # keeps e < 240 (fp8 max; overflow would be Inf, not
                    # saturation).
                    e8 = sb.tile([P, NKP, 2, T_LOC], F8, tag="e", bufs=2, name="e8")
                    for c in range(NKT):
                        s_ps = ps.tile([P, T_LOC], F32, tag="mm", bufs=4, name="s_ps")
                        nc.tensor.matmul(
                            s_ps[:],
                            kv_h[:, c // NTT, 0, c % NTT, :],
                            q_all[:, h, :],
                            start=True, stop=True,
                        )
                        nc.scalar.activation(
                            e8[:, c // 2, c % 2, :], s_ps[:],
                            mybir.ActivationFunctionType.Exp,
                            bias=ebias_t[:], scale=isq,
                        )
                    return e8

                # Q proj of head h is issued right before scores(h): attention
                # starts ~30us earlier (exp of head 0 overlaps the remaining Q
                # projections) and the per-head PE work (Q+scores+dn+av) then
                # slightly exceeds the scalar engine's exp, so neither stalls.
                attn_sb = sb.tile(
                    [P, NCP, 2, T_LOC], F8, tag="attn", bufs=1, name="attn_sb"
                )
                for h in range(NH):
                    q_proj(h)
                kv_h = load_head(0)
                e8 = scores_exp(0, kv_h)
                for h in range(NH):
                    if h + 1 < NH:
                        kv_h2 = load_head(h + 1)
                        e8_2 = scores_exp(h + 1, kv_h2)
                    dn_ps = ps.tile([P, T_LOC], F32, tag="dn", bufs=2, name="dn_ps")
                    for c in range(NKP):
                        nc.tensor.matmul(
                            dn_ps[:], ones8[:], e8[:, c, :, :],
                            start=(c == 0), stop=(c == NKP - 1), perf_mode=DR,
                        )
                    av_ps = ps.tile([P, T_LOC], F32, tag="acc", bufs=2, name="av_ps")
                    for c in range(NKP):
                        nc.tensor.matmul(
                            av_ps[:],
                            kv_h[:, c // 2, 1, (c % 2) * 2 : (c % 2) * 2 + 2, :],
                            e8[:, c, :, :],
                            start=(c == 0), stop=(c == NKP - 1), perf_mode=DR,
                        )
                    rcp = sb.tile([P, T_LOC], F32, tag="tmp", bufs=3, name="rcp")
                    nc.vector.reciprocal(rcp[:], dn_ps[:])
                    nc.vector.tensor_mul(attn_sb[:, h // 2, h % 2, :], av_ps[:], rcp[:])
                    if h + 1 < NH:
                        kv_h, e8 = kv_h2, e8_2

                # ---- o-proj (fp8 DR) + residual -> x2, with rmsnorm2's ----
                # ---- sum-of-squares interleaved per ot                   ----
                x2_sb = sb.tile([P, NCT, T_LOC], F32, tag="x2", bufs=1, name="x2_sb")
                ssq2 = ps.tile([P, T_LOC], F32, tag="dn", name="ssq2")
                for ot in range(NCT):
                    wo_t = sb.tile([P, NCP, 2, P], F8, tag="wqk", bufs=3, name="wo_t")
                    nc.sync.dma_start(
                        out=wo_t[:],
                        in_=woT_d[ot].rearrange("p (cp two f) -> p cp two f", two=2, f=P),
                    )
                    o_ps = ps.tile([P, T_LOC], F32, tag="mm", bufs=4, name="o_ps")
                    for cp in range(NCP):
                        nc.tensor.matmul(
                            o_ps[:], wo_t[:, cp, :, :], attn_sb[:, cp, :, :],
                            start=(cp == 0), stop=(cp == NCP - 1), perf_mode=DR,
                        )
                    nc.vector.tensor_add(x2_sb[:, ot, :], o_ps[:], x_sb[:, ot, :])
                    sq = sb.tile([P, T_LOC], BF16, tag="tmp", bufs=3, name="sq2")
                    nc.vector.tensor_mul(sq[:], x2_sb[:, ot, :], x2_sb[:, ot, :])
                    nc.tensor.matmul(
                        ssq2[:], ones_bf[:], sq[:], start=(ot == 0), stop=(ot == NCT - 1)
                    )

                # ---- rmsnorm2 tail -> x2n (bf16: the MLP must stay bf16) ----
                rms2 = sb.tile([P, T_LOC], F32, tag="tmp", bufs=3, name="rms2")
                nc.scalar.activation(
                    rms2[:], ssq2[:], mybir.ActivationFunctionType.Sqrt,
                    bias=eps_t[:], scale=1.0 / C,
                )
                rinv2 = sb.tile([P, T_LOC], F32, tag="rinv", bufs=2, name="rinv2")
                nc.vector.reciprocal(rinv2[:], rms2[:])
                x2n = sb.tile([P, NCP, 2, T_LOC], BF16, tag="xn", bufs=1, name="x2n")
                for ct in range(NCT):
                    eng = nc.vector if ct % 2 == 0 else nc.gpsimd
                    eng.tensor_mul(
                        x2n[:, ct // 2, ct % 2, :], x2_sb[:, ct, :], rinv2[:]
                    )

                # ---- MLP (bf16) in two halves of the intermediate dim ----
                for half in range(2):
                    h_sb = sb.tile(
                        [P, IHALF, T_LOC], BF16, tag="t32", bufs=1, name=f"h_sb{half}"
                    )
                    for ii in range(IHALF):
                        it = half * IHALF + ii
                        wg_t = sb.tile([P, C], BF16, tag="wgu", bufs=2, name="wg_t")
                        nc.sync.dma_start(out=wg_t[:], in_=wgT_d[it])
                        wu_t = sb.tile([P, C], BF16, tag="wgu", bufs=2, name="wu_t")
                        nc.sync.dma_start(out=wu_t[:], in_=wuT_d[it])
                        g_ps = ps.tile([P, T_LOC], F32, tag="mm", bufs=4, name="g_ps")
                        for ct in range(NCT):
                            nc.tensor.matmul(
                                g_ps[:], wg_t[:, ts(ct, P)], x2n[:, ct // 2, ct % 2, :],
                                start=(ct == 0), stop=(ct == NCT - 1),
                            )
                        u_ps = ps.tile([P, T_LOC], F32, tag="mm", bufs=4, name="u_ps")
                        for ct in range(NCT):
                            nc.tensor.matmul(
                                u_ps[:], wu_t[:, ts(ct, P)], x2n[:, ct // 2, ct % 2, :],
                                start=(ct == 0), stop=(ct == NCT - 1),
                            )
                        g_sb = sb.tile([P, T_LOC], BF16, tag="gs", bufs=2, name="g_sb")
                        nc.scalar.activation(
                            g_sb[:], g_ps[:], mybir.ActivationFunctionType.Silu
                        )
                        nc.vector.tensor_mul(h_sb[:, ii, :], u_ps[:], g_sb[:])

                    # down-proj (wd carries 64x to match the residual scale)
                    for ot in range(NCT):
                        wd_t = sb.tile([P, IHALF * P], BF16, tag="wd", bufs=2, name="wd_t")
                        nc.sync.dma_start(
                            out=wd_t[:], in_=wdT_d[ot][:, ts(half, IHALF * P)]
                        )
                        y_ps = ps.tile([P, T_LOC], F32, tag="acc", bufs=2, name="y_ps")
                        for ii in range(IHALF):
                            nc.tensor.matmul(
                                y_ps[:], wd_t[:, ts(ii, P)], h_sb[:, ii, :],
                                start=(ii == 0), stop=(ii == IHALF - 1),
                            )
                        nc.vector.tensor_add(x2_sb[:, ot, :], y_ps[:], x2_sb[:, ot, :])
                        if half == 1:
                            nc.sync.dma_start(
                                out=out_d[ts(ot, P), :], in_=x2_sb[:, ot, :]
                            )

    nc.compile()
    return nc


def _pack_inputs(x, w_ln1, wq, wk, wv, wo, w_ln2, wg, wu, wd):
    F8NP = ml_dtypes.float8_e4m3
    wq_eff = (wq * w_ln1[None, :]) * WS
    wk_eff = (wk * w_ln1[None, :]) * WS
    wv_eff = (wv * w_ln1[None, :]) * WS
    wg_eff = wg * w_ln2[None, :]
    wu_eff = wu * w_ln2[None, :]

    # V moving-operand layout: wv8[ci, ct, o] = wv_eff[o, 128*ct + ci]
    wv8 = np.ascontiguousarray(
        np.asarray(wv_eff).T.reshape(NCT, P, C).transpose(1, 0, 2)
    ).astype(F8NP)

    weights = {
        "wqT": _pack_lhsT(wq_eff, F8NP),
        "wkT": _pack_lhsT(wk_eff, F8NP),
        "wv8": wv8,
        "woT": _pack_lhsT(np.asarray(wo) * WS, F8NP),
        "wgT": _pack_lhsT(wg_eff, ml_dtypes.bfloat16),
        "wuT": _pack_lhsT(wu_eff, ml_dtypes.bfloat16),
        "wdT": _pack_lhsT(np.asarray(wd) * WS, ml_dtypes.bfloat16),
    }
    weights["ident"] = np.eye(P, dtype=np.float32)
    in_maps = []
    for core in range(N_CORES):
        b = core // GROUP
        t0 = (core % GROUP) * T_LOC
        x_loc = np.ascontiguousarray(
            np.asarray(x)[b, t0 : t0 + T_LOC, :].T * WS
        ).astype(np.float32)
        in_maps.append({"x": x_loc, **weights})
    return in_maps


def kernel(**inputs):
    if "nc" not in _CACHE:
        _CACHE["nc"] = _build_program()
    nc = _CACHE["nc"]
    in_maps = _pack_inputs(**inputs)
    res = run_bass_kernel_spmd(nc, in_maps, core_ids=list(range(N_CORES)))
    out = np.empty((B, T, C), dtype=np.float32)
    for core in range(N_CORES):
        b = core // GROUP
        t0 = (core % GROUP) * T_LOC
        out[b, t0 : t0 + T_LOC, :] = res.results[core]["out"].T * (1.0 / WS)
    return out

